# revision 12
# baseline (speedup 1.0000x reference)
"""Trainium2 Bass/Tile kernel for ExtAttentionPool (nn_ExtAttentionPool).

Math (per sample b):
    S[u, o]  = sum_d L[u, d] * W[o, d]
    E[o, u]  = exp(S[u,o]/O + b[o]/O)          (softmax numerator over u)
    Z[o]     = sum_u E[o, u]
    OUT[o,t] = (1/Z[o]) * sum_c E[o, c] * L[t, c]
    result row b = OUT flattened (O-major), shape (O*T,)

Sharding: data-parallel over batch B=16 across 8 cores (2 samples/core).

Both matmuls contract over logits' D axis, so logits is transposed on-chip.
The transpose is done as a REGULAR bf16 matmul against an identity moving
operand (out = L_chunk.T @ I): unlike PE transpose-mode this engages the
fast-weight-load path and counts as PE activity for the HAM clock gate.
Logits are cast f32->bf16 inline in the SWDGE DMA load; accumulation stays
fp32 in PSUM. The kernel opens with ~36 identity matmuls so the PE clock
is already at 2.4 GHz when the first data arrives. The second matmul's two
512-wide halves run concurrently in different PE column groups
(tile_position), and the 1/Z softmax normalization is folded into the
E-transpose by using diag(1/Z) as the transpose's moving operand.
"""

import numpy as np
from contextlib import ExitStack

import concourse.bass as bass
import concourse.mybir as mybir
import concourse.tile as tile
from concourse import bacc
from concourse.bass_utils import run_bass_kernel_spmd
from concourse.masks import make_identity

F32 = mybir.dt.float32
BF16 = mybir.dt.bfloat16

N_CORES = 8
B_FULL = 16


def build_nc(b_per=2, T=1024, D=1024, O=10, warmup_mms=36):
    """Build the per-core Bass program (bf16 compute). Same on all 8 cores."""
    P = 128
    NT = T // P            # 128-row t-blocks
    ND = D // P            # 128-col d-blocks
    NH = max(1, T // 512)  # psum-width output slices
    HW = min(T, 512)
    # per-sample DMA chunk plans (in 128-row blocks): small first chunks so
    # the PE starts early; small last chunks so the tail dependency is short.
    if NT == 8:
        plans = [[1, 1, 2, 2, 2], [2, 2, 2, 1, 1]]
    else:
        plans = [[1] * NT, [1] * NT]

    nc = bacc.Bacc(
        "TRN2", target_bir_lowering=False, debug=False, enable_asserts=False
    )
    logits = nc.dram_tensor("logits", (b_per, T, D), F32, kind="ExternalInput").ap()
    w_in = nc.dram_tensor("W", (O, D), F32, kind="ExternalInput").ap()
    b_in = nc.dram_tensor("b", (O,), F32, kind="ExternalInput").ap()
    out = nc.dram_tensor("out", (b_per, O * T), F32, kind="ExternalOutput").ap()

    with tile.TileContext(nc) as tc, ExitStack() as ctx:
        singles = ctx.enter_context(tc.tile_pool(name="singles", bufs=1))
        lr_pool = ctx.enter_context(tc.tile_pool(name="lr", bufs=3))
        lt_pool = ctx.enter_context(tc.tile_pool(name="lt", bufs=2))
        e_pool = ctx.enter_context(tc.tile_pool(name="e", bufs=2))
        z_pool = ctx.enter_context(tc.tile_pool(name="z", bufs=2))
        osb_pool = ctx.enter_context(tc.tile_pool(name="osb", bufs=2))
        slab_ps = ctx.enter_context(tc.tile_pool(name="slab", bufs=2, space="PSUM"))
        s_ps = ctx.enter_context(tc.tile_pool(name="sps", bufs=2, space="PSUM"))
        o_ps = ctx.enter_context(tc.tile_pool(name="ops", bufs=1, space="PSUM"))
        et_ps = ctx.enter_context(tc.tile_pool(name="etps", bufs=1, space="PSUM"))

        # --- identity first (PE warmup depends on it) ---
        ident = singles.tile([P, P], BF16)
        make_identity(nc, ident)

        # --- weights / bias loads ---
        w_sb = singles.tile([O, D], BF16)
        nc.gpsimd.dma_start(out=w_sb, in_=w_in)  # SWDGE cast f32->bf16
        b_sb = singles.tile([O, 1], F32)
        nc.sync.dma_start(out=b_sb, in_=b_in.rearrange("(o u) -> o u", u=1))
        bias01 = singles.tile([O, 1], F32)
        nc.scalar.activation(
            out=bias01, in_=b_sb,
            func=mybir.ActivationFunctionType.Copy, scale=1.0 / O,
        )

        # --- PE warmup: identity matmuls to lift the HAM clock gate ---
        warm = slab_ps.tile([P, 4 * P], F32, tag="slab")
        for i in range(warmup_mms):
            k = i % 4
            nc.tensor.matmul(
                warm[:, k * P : (k + 1) * P], lhsT=ident, rhs=ident,
                start=True, stop=True,
            )

        # WT[dp, c, o] = W[o, 128c+dp]  (regular-matmul transpose)
        wt_stage = et_ps.tile([P, ND, O], F32, tag="etps")
        for c in range(ND):
            nc.tensor.matmul(
                wt_stage[:, c, :],
                lhsT=w_sb[:, c * P : (c + 1) * P],
                rhs=ident[:O, :O],
                start=True, stop=True,
            )
        wt_sb = singles.tile([P, ND, O], BF16)
        nc.vector.tensor_copy(wt_sb, wt_stage)

        i_copy = [0]

        def phase_load_transpose(s):
            """Cast-load logits[s], build LT[dp, c, t] = L[t, 128c+dp] (bf16)."""
            lt = lt_pool.tile([P, ND, T], BF16, tag="lt")
            r = 0
            for rj in plans[s]:
                lr = lr_pool.tile([P, rj, D], BF16, tag="lr")
                nc.gpsimd.dma_start(
                    out=lr[:, :rj, :],
                    in_=logits[
                        s, r * P : (r + rj) * P, :
                    ].rearrange("(j p) d -> p j d", p=P),
                )
                for j in range(rj):
                    for g in range(ND // 4):
                        slab = slab_ps.tile([P, 4 * P], F32, tag="slab")
                        for k in range(4):
                            c = 4 * g + k
                            nc.tensor.matmul(
                                slab[:, k * P : (k + 1) * P],
                                lhsT=lr[:, j, c * P : (c + 1) * P],
                                rhs=ident,
                                start=True, stop=True,
                            )
                        dst = lt[
                            :, 4 * g : 4 * g + 4, (r + j) * P : (r + j + 1) * P
                        ]
                        # split PSUM->SBUF cast-copies between DVE and ACT
                        if i_copy[0] % 2 == 0:
                            nc.vector.tensor_copy(dst, slab)
                        else:
                            nc.scalar.activation(
                                out=dst, in_=slab,
                                func=mybir.ActivationFunctionType.Copy,
                            )
                        i_copy[0] += 1
                r += rj
            return lt

        def phase_mm1(s, lt):
            s_tiles = []
            for h in range(NH):
                sp = s_ps.tile([O, HW], F32, tag="sps")
                for c in range(ND):
                    nc.tensor.matmul(
                        sp,
                        lhsT=wt_sb[:, c, :],
                        rhs=lt[:, c, h * HW : (h + 1) * HW],
                        start=(c == 0),
                        stop=(c == ND - 1),
                    )
                s_tiles.append(sp)
            return s_tiles

        def phase_softmax(s, s_tiles):
            e_sb = e_pool.tile([O, T], BF16, tag="e")
            zparts = z_pool.tile([O, NH], F32, tag="z")
            for h in range(NH):
                nc.scalar.activation(
                    out=e_sb[:, h * HW : (h + 1) * HW],
                    in_=s_tiles[h],
                    func=mybir.ActivationFunctionType.Exp,
                    scale=1.0 / O,
                    bias=bias01,
                    accum_out=zparts[:, h : h + 1],
                )
            zsum = z_pool.tile([O, 1], F32, tag="zs")
            if NH == 2:
                nc.vector.tensor_add(zsum, zparts[:, 0:1], zparts[:, 1:2])
            elif NH == 1:
                nc.vector.tensor_copy(zsum, zparts)
            else:
                nc.vector.reduce_sum(zsum, zparts, axis=mybir.AxisListType.X)
            rz = z_pool.tile([O, 1], F32, tag="rz")
            nc.vector.reciprocal(rz, zsum)
            # diag(1/Z) in bf16: moving operand for the E-transpose, folding
            # the softmax normalization into the transpose matmul.
            diag = z_pool.tile([O, O], BF16, tag="diag")
            nc.vector.tensor_scalar_mul(diag, ident[:O, :O], rz)
            return e_sb, diag

        def phase_et(s, e_sb, diag):
            """EC[cp, c, o] = E[o, 128c+cp] / Z[o]  (transpose x diag(1/Z))."""
            et_stage = et_ps.tile([P, ND, O], F32, tag="etps")
            for c in range(ND):
                nc.tensor.matmul(
                    et_stage[:, c, :],
                    lhsT=e_sb[:, c * P : (c + 1) * P],
                    rhs=diag,
                    start=True, stop=True,
                )
            ec = e_pool.tile([P, ND, O], BF16, tag="ec")
            nc.vector.tensor_copy(ec, et_stage)
            return ec

        def phase_mm2_fin(s, lt, ec):
            # both 512-halves run concurrently in PE column groups 0 and 1
            # (separate PSUM banks; out base_partition matches tile_position)
            op0 = o_ps.tile([O, HW], F32, tag="ops0")
            op1 = o_ps.tile([42, HW], F32, tag="ops1")
            outs = [op0, op1[32 : 32 + O, :]]
            for c in range(ND):
                for h in range(NH):
                    nc.tensor.matmul(
                        outs[h],
                        lhsT=ec[:, c, :],
                        rhs=lt[:, c, h * HW : (h + 1) * HW],
                        start=(c == 0),
                        stop=(c == ND - 1),
                        tile_position=(0, 32 * h),
                    )
            o_sb = osb_pool.tile([42, T], F32, tag="osb")
            views = [o_sb[0:O, 0:HW], o_sb[32 : 32 + O, HW:T]]
            for h in range(NH):
                nc.scalar.activation(
                    out=views[h], in_=outs[h],
                    func=mybir.ActivationFunctionType.Copy,
                )
                nc.sync.dma_start(
                    out=out[s].rearrange("(o t) -> o t", o=O)[
                        :, h * HW : (h + 1) * HW
                    ],
                    in_=views[h],
                )

        # software-pipelined schedule over the per-core samples
        lt0 = phase_load_transpose(0)
        st0 = phase_mm1(0, lt0)
        e0, diag0 = phase_softmax(0, st0)
        prev = (lt0, e0, diag0)
        for s in range(1, b_per):
            lt_n = phase_load_transpose(s)
            st_n = phase_mm1(s, lt_n)
            lt_p, e_p, diag_p = prev
            ec_p = phase_et(s - 1, e_p, diag_p)
            phase_mm2_fin(s - 1, lt_p, ec_p)
            e_n, diag_n = phase_softmax(s, st_n)
            prev = (lt_n, e_n, diag_n)
        lt_l, e_l, diag_l = prev
        ec_l = phase_et(b_per - 1, e_l, diag_l)
        phase_mm2_fin(b_per - 1, lt_l, ec_l)

    nc.compile()
    return nc


_NC = None
TRACE = False
LAST_RESULT = None
BUILD_KWARGS = {}


def _get_nc():
    global _NC
    if _NC is None:
        _NC = build_nc(**BUILD_KWARGS)
    return _NC


def kernel(logits, decision, W, b):
    """Full-input entry point: shards batch over 8 cores, returns (16, 10240)."""
    global LAST_RESULT
    logits = np.asarray(logits, dtype=np.float32)
    W = np.asarray(W, dtype=np.float32)
    b = np.asarray(b, dtype=np.float32)
    nc = _get_nc()
    bp = B_FULL // N_CORES
    in_maps = [
        {"logits": np.ascontiguousarray(logits[i * bp : (i + 1) * bp]), "W": W, "b": b}
        for i in range(N_CORES)
    ]
    res = run_bass_kernel_spmd(nc, in_maps, core_ids=list(range(N_CORES)), trace=TRACE)
    LAST_RESULT = res
    return np.concatenate([res.results[i]["out"] for i in range(N_CORES)], axis=0)
